# revision 64
# baseline (speedup 1.0000x reference)
"""Trainium2 Bass kernel for the ConvNet problem.

v3 design (all-bf16 matmul path, fp32 PSUM accumulation):
  - Host ships signal only, as bf16 [B+1, 150] (padded row).  Per
    super-tile (2048 samples) two DMA-XBAR transposes load feature-major
    bf16 tiles straight from DRAM: xA (feats 0..127) and xB (feats
    128..146 + harmless overrun, 32 rows).  x^2 tiles are computed
    on-device (DVE/Pool elementwise multiply) -- halves HBM traffic vs
    shipping signal^2 from host.
  - Windowed sums (banded matmuls), std = sqrt((s2 - s'^2)/9).
  - conv1 is emitted as three band-replicated output chunks (h1 rows
    0..74 / 60..139 / 125..189) so that each conv2 output chunk of 128
    features needs a single K-chunk: conv1 3 matmuls + conv2 3 matmuls
    (vs 2 + 6 for the dense 2-chunk layout).
  - fc1 (3 K-chunks x 2 M-chunks), fc2, pamap via weight streaming,
    log_softmax in a second phase (Exp/Ln table).
  - PSUM tags: pS / pS2 (1 bank each, reused by fc2 / pamap), pH1
    (3 banks, conv1 + fc1), pH2 (3 banks, conv2) = 8 banks total.
  - Drains are ReLUs spread across Act/DVE/Pool for engine balance
    (biases in this problem are structurally zero; a bias-correct
    fallback program is built if any bias is nonzero).

Sharding: pure data parallelism, batch split 8 ways across NeuronCores.
"""

import math
import os

import numpy as np

# ---------------------------------------------------------------------------
# Problem constants (hardcoded; kernel.py must be self-contained)
# ---------------------------------------------------------------------------
B_TOTAL, L, C = 131072, 50, 3
WIN = 10
NCORES = 8
B_CORE = B_TOTAL // NCORES          # 16384
G_SUPER = 16                        # samples per partition per super-tile
SUPER = 128 * G_SUPER               # 2048 samples per super-tile
N_SUPER_FULL = B_CORE // SUPER      # 8
NSUB = 256                          # samples per sub-tile (matmul N)
SUBQ = SUPER // NSUB                # 4 sub-tiles per super-tile
F_IN = L * C                        # 150
F_USE = 147                         # features actually consumed
F_STD = 120                         # 40 windows x 3 channels
F_C1 = 190                          # 38 x 5
F_C2 = 360                          # 36 x 10
F_FC1 = 256
F_FC2 = 64
F_OUT = 12

# conv1 band-replica chunks (h1 row ranges, in 190-row coordinates) chosen
# so conv2 output chunk m (features 128m..) reads h1 rows from replica m
# starting at partition 0.
C1_CH = ((0, 75), (60, 140), (125, 190))      # (lo, hi) -> rows lo..hi-1
C2_MO = (0, 128, 256, 360)                    # conv2 output chunk offsets
FC1_K = (128, 128, 104)                       # fc1 K-chunk sizes

# debug knob (harness never sets this; default = full problem)
_N_SUPER = int(os.environ.get("ATRN_NSUPER", str(N_SUPER_FULL)))


# ---------------------------------------------------------------------------
# Tile drain patch: walrus in this container rejects >2 sem waits on a
# CTRL-class (Drain) instruction.  Spread the end-of-kernel global-clock waits
# across per-proc SP nops (one sem each) before an unadorned drain.
# ---------------------------------------------------------------------------
def _install_drain_patch():
    import concourse.tile as tile
    from concourse.tile_scheduler import N_PROCS
    from concourse.vector_clock import ScopedClock, VectorClock

    if getattr(tile.TileContext, "_drain_patch_installed", False):
        return

    def _patched_drain_and_barrier(self, tick_clock, wait_clock):
        nc = self.nc
        gc = tick_clock.global_clock
        for p in range(N_PROCS):
            if gc[p] <= 0:
                continue
            v = [0] * N_PROCS
            v[p] = gc[p]
            nop = nc.sync.nop()
            wait_clock.add_sem_waits(nop.ins, ScopedClock({None: VectorClock(v)}))
        nc.sync.drain()
        nc.all_engine_barrier()
        assert self.sems is not None
        popped = nc._tile_sem_poison_stack.pop()
        assert popped is self._sem_poison
        nc.clear_and_free_semaphores(list(self.sems.allocated().values()))
        nc.all_engine_barrier()

    tile.TileContext._drain_and_barrier = _patched_drain_and_barrier
    tile.TileContext._drain_patch_installed = True


def _wait_cap(ins):
    # This walrus build rejects >1 sem wait on engine instructions.
    return 1


def _split_excess_waits(nc):
    """Hoist excess sem waits onto same-engine nops inserted just before."""
    from concourse import mybir

    ctr = 0
    for f in nc.m.functions:
        for blk in f.blocks:
            il = blk.instructions
            i = 0
            while i < len(il):
                ins = il[i]
                si = ins.sync_info
                cap = _wait_cap(ins)
                if si is not None and len(si.on_wait) > cap:
                    waits = list(si.on_wait)
                    extra, keep = waits[:-cap], waits[-cap:]
                    for w in extra:
                        ctr += 1
                        nop = mybir.InstNoOp(name=f"waitsplit-{ctr}",
                                             ins=[], outs=[])
                        nop.engine = ins.engine
                        nop.sync_info = type(si)(on_wait=[w], on_update=[])
                        nc.register_instruction(nop, overwrite=True)
                        il.insert(i, nop)
                        i += 1
                    ins.sync_info = type(si)(on_wait=keep,
                                             on_update=list(si.on_update))
                i += 1


# ---------------------------------------------------------------------------
# Host-side weight preprocessing
#   wb16: [128, W16] bf16 matmul weights
#   wb32: [128, W32] f32 biases (fallback variant) + bp for phase B
# ---------------------------------------------------------------------------
class _BlobLayout:
    def __init__(self):
        self.cols = 0
        self.slots = {}

    def add(self, name, rows, cols):
        self.slots[name] = (self.cols, rows, cols)
        self.cols += cols
        return self.slots[name]


_L16 = _BlobLayout()
_L16.add("sA_a", 128, F_STD)    # windowed-sum (scaled 1/sqrt(10)) rows 0..127
_L16.add("sA_b", 19, F_STD)     # rows 128..146
_L16.add("s2_a", 128, F_STD)    # unscaled (exact 1.0 in bf16)
_L16.add("s2_b", 19, F_STD)
for j in range(3):
    _L16.add(f"c1r{j}", F_STD, 80)  # conv1 band replica j (zero-padded to 80)
for m in range(3):
    lo, hi = C1_CH[m]
    _L16.add(f"c2m{m}", hi - lo, 128)  # chunk2 zero-padded 104->128
for k in range(3):
    for m in range(2):
        _L16.add(f"f1_{k}_{m}", FC1_K[k], 128)
for k in range(2):
    _L16.add(f"f2_{k}", 128, F_FC2)
_L16.add("wp", F_FC2, F_OUT)    # pamap rhs (weight-streaming)
W16 = _L16.cols

_L32 = _BlobLayout()
_L32.add("b1", 80, 3)           # conv1 bias per (partition, replica chunk)
_L32.add("b2", 128, 3)          # conv2 bias per (partition, chunk)
_L32.add("b3", 128, 2)          # fc1 bias
_L32.add("b4", F_FC2, 1)        # fc2 bias
_L32.add("bp", 128, F_OUT)      # pamap bias replicated across partitions
W32 = _L32.cols


def _build_blobs(conv1_w, conv1_b, conv2_w, conv2_b, fc1_w, fc1_b, fc2_w,
                 fc2_b, pamap_w, pamap_b):
    import ml_dtypes
    b16 = np.zeros((128, W16), ml_dtypes.bfloat16)
    b32 = np.zeros((128, W32), np.float32)

    def put16(name, arr):
        off, rows, cols = _L16.slots[name]
        assert arr.shape == (rows, cols), (name, arr.shape, (rows, cols))
        b16[:rows, off:off + cols] = arr.astype(ml_dtypes.bfloat16)

    def put32(name, arr):
        off, rows, cols = _L32.slots[name]
        assert arr.shape == (rows, cols), (name, arr.shape, (rows, cols))
        b32[:rows, off:off + cols] = arr.astype(np.float32)

    # windowed sums: s'[3l+c] = (1/sqrt(10)) * sum_k x[3(l+k)+c]
    A = np.zeros((F_USE, F_STD), np.float32)
    for m in range(F_STD):
        l, c = divmod(m, 3)
        for k in range(WIN):
            A[3 * (l + k) + c, m] = 1.0
    put16("sA_a", A[:128] / math.sqrt(10.0))
    put16("sA_b", A[128:] / math.sqrt(10.0))
    put16("s2_a", A[:128])
    put16("s2_b", A[128:])

    # conv1 as dense [in 120, out 190]
    M1 = np.zeros((F_STD, F_C1), np.float32)
    for t in range(38):
        for o in range(5):
            for k in range(3):
                for i in range(3):
                    M1[3 * (t + k) + i, 5 * t + o] = conv1_w[o, i, k]
    for j, (lo, hi) in enumerate(C1_CH):
        blk = np.zeros((F_STD, 80), np.float32)
        blk[:, :hi - lo] = M1[:, lo:hi]
        put16(f"c1r{j}", blk)

    # conv2 as dense [in 190, out 360]; chunk m reads h1 replica m
    M2 = np.zeros((F_C1, F_C2), np.float32)
    for t in range(36):
        for o in range(10):
            for k in range(3):
                for i in range(5):
                    M2[5 * (t + k) + i, 10 * t + o] = conv2_w[o, i, k]
    for m in range(3):
        lo, hi = C1_CH[m]
        blk = np.zeros((hi - lo, 128), np.float32)
        blk[:, :C2_MO[m + 1] - C2_MO[m]] = M2[lo:hi, C2_MO[m]:C2_MO[m + 1]]
        # band check: all nonzeros of these output columns live in rows lo..hi
        assert np.all(M2[:lo, C2_MO[m]:C2_MO[m + 1]] == 0)
        assert np.all(M2[hi:, C2_MO[m]:C2_MO[m + 1]] == 0)
        put16(f"c2m{m}", blk)

    # fc1 [360, 256]; K-chunks match conv2 output chunks (128/128/104)
    F1 = fc1_w.T.astype(np.float32)
    for k in range(3):
        for m in range(2):
            put16(f"f1_{k}_{m}",
                  F1[C2_MO[k]:C2_MO[k + 1], m * 128:(m + 1) * 128])
    F2 = fc2_w.T.astype(np.float32)          # [256, 64]
    for k in range(2):
        put16(f"f2_{k}", F2[k * 128:(k + 1) * 128])
    put16("wp", pamap_w.T.astype(np.float32))  # [64, 12]

    # biases (fallback variant only; bp always used in phase B)
    b1 = np.zeros(F_C1, np.float32)
    for t in range(38):
        for o in range(5):
            b1[5 * t + o] = conv1_b[o]
    b1p = np.zeros((80, 3), np.float32)
    for j, (lo, hi) in enumerate(C1_CH):
        b1p[:hi - lo, j] = b1[lo:hi]
    put32("b1", b1p)
    b2 = np.zeros(F_C2, np.float32)
    for t in range(36):
        for o in range(10):
            b2[10 * t + o] = conv2_b[o]
    b2p = np.zeros((128, 3), np.float32)
    for m in range(3):
        mo = C2_MO[m + 1] - C2_MO[m]
        b2p[:mo, m] = b2[C2_MO[m]:C2_MO[m + 1]]
    put32("b2", b2p)
    put32("b3", np.stack([fc1_b[:128], fc1_b[128:]], axis=1))
    put32("b4", fc2_b[:, None])
    put32("bp", np.tile(pamap_b.astype(np.float32)[None, :], (128, 1)))
    return b16, b32


# ---------------------------------------------------------------------------
# Bass program
# ---------------------------------------------------------------------------
_PROGRAMS = {}


def _w16(w, name):
    off, rows, cols = _L16.slots[name]
    return w[0:rows, off:off + cols]


def _w32(w, name):
    off, rows, cols = _L32.slots[name]
    return w[0:rows, off:off + cols]


def _build_program(n_super, use_bias):
    import contextlib

    import concourse.bass as bass
    import concourse.tile as tile
    from concourse import mybir

    _install_drain_patch()
    f32 = mybir.dt.float32
    bf16 = mybir.dt.bfloat16
    AF = mybir.ActivationFunctionType
    ALU = mybir.AluOpType

    b_core = n_super * SUPER
    nc = bass.Bass("TRN2", target_bir_lowering=False, debug=False,
                   num_devices=NCORES)
    # +1 padded row: the feats-128..159 transpose window of the last sample
    # runs into the pad row instead of out of bounds.
    sig = nc.dram_tensor("sig", [b_core + 1, F_IN], bf16,
                         kind="ExternalInput")
    wb16 = nc.dram_tensor("wb16", [128, W16], bf16, kind="ExternalInput")
    wb32 = nc.dram_tensor("wb32", [128, W32], f32, kind="ExternalInput")
    out = nc.dram_tensor("out", [b_core, F_OUT], f32, kind="ExternalOutput")

    with tile.TileContext(nc) as tc:
        with contextlib.ExitStack() as ctx:
            singles = ctx.enter_context(tc.tile_pool(name="singles", bufs=1))
            xsup = ctx.enter_context(tc.tile_pool(name="xsup", bufs=2))
            sbx = ctx.enter_context(tc.tile_pool(name="sbx", bufs=3))
            sbh = ctx.enter_context(tc.tile_pool(name="sbh", bufs=3))
            psA = ctx.enter_context(tc.tile_pool(name="psA", bufs=1,
                                                 space="PSUM"))
            outp = ctx.enter_context(tc.tile_pool(name="outp", bufs=2))

            w16 = singles.tile([128, W16], bf16)
            nc.sync.dma_start(out=w16, in_=wb16[:, :])
            w32 = singles.tile([128, W32], f32)
            nc.scalar.dma_start(out=w32, in_=wb32[:, :])
            lgpool = ctx.enter_context(tc.tile_pool(name="lgp", bufs=1))
            logits_all = lgpool.tile([128, n_super * G_SUPER * F_OUT], f32,
                                     name="logits_all", tag="lg")

            # column n of sub-tile q holds sample q*512 + j*128 + p after the
            # weight-streaming pamap matmul; phase-B g index is (q j).
            out_v = out.rearrange("(T q j p) o -> T p (q j) o",
                                  q=SUBQ, j=NSUB // 128, p=128)

            def relu_drain(eng, dst, src, bias=None):
                if bias is not None:
                    eng_v = nc.vector if eng is nc.scalar else eng
                    eng_v.tensor_scalar(out=dst, in0=src, scalar1=bias,
                                        scalar2=0.0, op0=ALU.add, op1=ALU.max)
                elif eng is nc.scalar:
                    nc.scalar.activation(out=dst, in_=src, func=AF.Relu)
                else:
                    eng.tensor_scalar_max(out=dst, in0=src, scalar1=0.0)

            import concourse.bass as bass_mod

            def tr_src(t, T, coff, width):
                full = t[:, :]
                return bass_mod.AP(
                    tensor=full.tensor, offset=T * SUPER * F_IN + coff,
                    ap=[[F_IN, SUPER], [1, width]])

            n_sub = n_super * SUBQ
            xtiles = {}                 # super index -> (xA, xB, qA, qB)
            state = {}                  # cycle -> dict of live tiles

            def tr_src_at(t, base, coff, width, n):
                full = t[:, :]
                return bass_mod.AP(
                    tensor=full.tensor, offset=base * F_IN + coff,
                    ap=[[F_IN, n], [1, width]])

            def emit_transposes(T):
                # XBAR transpose needs src free size to be a multiple of 128,
                # so xB reads a full 128-wide window (overrun lands in the
                # next sample / pad row; only rows 0..18 are consumed).
                xA = xsup.tile([128, SUPER], bf16, tag="xA")
                xB = xsup.tile([128, SUPER], bf16, tag="xB")
                # chunked for super 0 so the pipeline's first windows are not
                # gated on a whole-super transpose
                nchunk = 8 if T == 0 else 1
                cw = SUPER // nchunk
                for c in range(nchunk):
                    base = T * SUPER + c * cw
                    nc.sync.dma_start_transpose(
                        out=xA[:, c * cw:(c + 1) * cw],
                        in_=tr_src_at(sig, base, 0, 128, cw))
                    nc.sync.dma_start_transpose(
                        out=xB[:, c * cw:(c + 1) * cw],
                        in_=tr_src_at(sig, base, 128, 128, cw))
                xtiles[T] = (xA, xB)

            def emit_stage0(v):
                # windowed sums + std chain for cycle v
                xA, xB = xtiles[v // SUBQ]
                cs = slice((v % SUBQ) * NSUB, (v % SUBQ + 1) * NSUB)
                st = state[v] = {}
                # x^2 on-device, one sub-tile slice at a time (saves shipping
                # signal^2 over HBM without a serial whole-super bubble)
                qa = sbx.tile([128, NSUB], bf16, tag="qa")
                nc.gpsimd.tensor_mul(out=qa, in0=xA[:, cs], in1=xA[:, cs])
                qb = sbx.tile([32, NSUB], bf16, tag="qb")
                nc.gpsimd.tensor_mul(out=qb, in0=xB[0:32, cs],
                                     in1=xB[0:32, cs])
                # s and s2 packed in one 2-bank PSUM tile: the pS ring is
                # stage0-only, so next window's stage0 never waits on drains
                sps = psA.tile([128, 2, NSUB], f32, tag="pS")
                nc.tensor.matmul(sps[0:F_STD, 0], _w16(w16, "sA_a"),
                                 xA[:, cs], start=True, stop=False)
                nc.tensor.matmul(sps[0:F_STD, 0], _w16(w16, "sA_b"),
                                 xB[0:19, cs], start=False, stop=True)
                nc.tensor.matmul(sps[0:F_STD, 1], _w16(w16, "s2_a"), qa,
                                 start=True, stop=False)
                nc.tensor.matmul(sps[0:F_STD, 1], _w16(w16, "s2_b"),
                                 qb[0:19], start=False, stop=True)
                t_sb = sbx.tile([F_STD, NSUB], f32, tag="t")
                # hw verifier: only ONE non-scalar PSUM input per instruction,
                # so the square must be the single-input Act Square
                nc.scalar.activation(out=t_sb, in_=sps[0:F_STD, 0],
                                     func=AF.Square)
                u_sb = sbx.tile([F_STD, NSUB], f32, tag="u")
                nc.vector.tensor_sub(out=u_sb, in0=sps[0:F_STD, 1], in1=t_sb)
                std = sbx.tile([F_STD, NSUB], bf16, tag="std")
                nc.scalar.activation(out=std, in_=u_sb, func=AF.Sqrt,
                                     scale=1.0 / 9.0)
                st["std"] = std

            def emit_conv1(v):
                st = state[v]
                std = st.pop("std")
                h1_ps = psA.tile([128, 3, NSUB], f32, tag="pH1")
                for j in range(3):
                    nc.tensor.matmul(h1_ps[0:80, j],
                                     _w16(w16, f"c1r{j}"), std)
                h1 = sbh.tile([80, 3, NSUB], bf16, tag="h1")
                if use_bias:
                    for j in range(3):
                        relu_drain(nc.scalar, h1[:, j], h1_ps[0:80, j],
                                   _w32(w32, "b1")[:, j:j + 1])
                else:
                    relu_drain(nc.scalar, h1, h1_ps[0:80])
                st["h1"] = h1

            def emit_conv2(v):
                st = state[v]
                h1 = st.pop("h1")
                h2_ps = psA.tile([128, 3, NSUB], f32, tag="pH2")
                for m in range(3):
                    lo, hi = C1_CH[m]
                    nc.tensor.matmul(h2_ps[:, m], _w16(w16, f"c2m{m}"),
                                     h1[0:hi - lo, m])
                h2 = sbh.tile([128, 3, NSUB], bf16, tag="h2")
                if use_bias:
                    for m in range(3):
                        relu_drain(nc.vector, h2[:, m], h2_ps[:, m],
                                   _w32(w32, "b2")[:, m:m + 1])
                else:
                    relu_drain(nc.vector, h2, h2_ps)
                st["h2"] = h2

            def emit_fc1(v):
                st = state[v]
                h2 = st.pop("h2")
                h3_ps = psA.tile([128, 2, NSUB], f32, tag="pH3")
                for m in range(2):
                    for k in range(3):
                        nc.tensor.matmul(h3_ps[:, m],
                                         _w16(w16, f"f1_{k}_{m}"),
                                         h2[0:FC1_K[k], k], start=(k == 0),
                                         stop=(k == 2))
                h3 = sbh.tile([128, 2, NSUB], bf16, tag="h3")
                if use_bias:
                    for m in range(2):
                        relu_drain(nc.vector, h3[:, m], h3_ps[:, m],
                                   _w32(w32, "b3")[:, m:m + 1])
                else:
                    relu_drain(nc.scalar, h3[:, 0], h3_ps[:, 0])
                    relu_drain(nc.vector, h3[:, 1], h3_ps[:, 1])
                st["h3"] = h3

            def emit_tail(v):
                st = state.pop(v)
                h3 = st["h3"]
                # h4 and logits share one PSUM tile on the pH2 ring: the tail
                # gates conv2(v+3) (mid-pipeline) instead of stage0 (head).
                tp = psA.tile([128, 2, NSUB], f32, tag="pT", bufs=2)
                h4_ps = tp[0:F_FC2, 0]
                for k in range(2):
                    nc.tensor.matmul(h4_ps, _w16(w16, f"f2_{k}"),
                                     h3[:, k], start=(k == 0), stop=(k == 1))
                h4 = sbh.tile([F_FC2, NSUB], bf16, tag="h4")
                relu_drain(nc.scalar, h4, h4_ps,
                           _w32(w32, "b4") if use_bias else None)
                nj = NSUB // 128
                lg_ps = tp[:, 1, 0:nj * F_OUT]
                for j in range(nj):
                    nc.tensor.matmul(
                        lg_ps[:, j * F_OUT:(j + 1) * F_OUT],
                        h4[:, j * 128:(j + 1) * 128], _w16(w16, "wp"),
                        start=True, stop=True)
                nc.vector.tensor_copy(
                    out=logits_all[:, v * nj * F_OUT:(v + 1) * nj * F_OUT],
                    in_=lg_ps)

            # software pipeline: stage k of cycle v-k all emitted in window v.
            # Old cycles come first so each engine's in-order queue leads with
            # work whose inputs finished in earlier windows.
            emit_transposes(0)
            for v in range(n_sub + 4):
                if v % SUBQ == 0 and (v // SUBQ) + 1 < n_super:
                    emit_transposes(v // SUBQ + 1)
                if v < n_sub:
                    emit_stage0(v)
                if 0 <= v - 1 < n_sub:
                    emit_conv1(v - 1)
                if 0 <= v - 2 < n_sub:
                    emit_conv2(v - 2)
                if 0 <= v - 3 < n_sub:
                    emit_fc1(v - 3)
                if 0 <= v - 4 < n_sub:
                    emit_tail(v - 4)

            # ---------- phase B: log-softmax (exp/ln table set) ----------
            tc.no_sync_barrier()
            PAIR = 2
            n_pg = PAIR * G_SUPER
            w_bp = _w32(w32, "bp")                       # [128, 12]
            bp3d = bass_mod.AP(tensor=w_bp.tensor, offset=w_bp.offset,
                               ap=[w_bp.ap[0], [0, n_pg], w_bp.ap[1]])
            out_v2 = out.rearrange("(U g p) o -> U p g o", g=n_pg, p=128)
            for U in range(n_super // PAIR):
                chunk = logits_all[:, U * n_pg * F_OUT:
                                   (U + 1) * n_pg * F_OUT]
                ch3 = chunk.rearrange("p (g o) -> p g o", o=F_OUT)
                lb = outp.tile([128, n_pg, F_OUT], f32, tag="lb")
                nc.gpsimd.tensor_tensor(out=lb, in0=ch3, in1=bp3d, op=ALU.add)
                e = outp.tile([128, n_pg, F_OUT], f32, tag="e")
                nc.scalar.activation(out=e, in_=lb, func=AF.Exp)
                ssum = outp.tile([128, n_pg], f32, tag="ss")
                nc.vector.tensor_reduce(out=ssum, in_=e,
                                        axis=mybir.AxisListType.X, op=ALU.add)
                lse = outp.tile([128, n_pg], f32, tag="lse")
                nc.scalar.activation(out=lse, in_=ssum, func=AF.Ln)
                lse3 = bass_mod.AP(tensor=lse.tensor, offset=lse.offset,
                                   ap=[lse.ap[0], lse.ap[1], [0, F_OUT]])
                ot = outp.tile([128, n_pg, F_OUT], f32, tag="ot")
                nc.vector.tensor_tensor(out=ot, in0=lb, in1=lse3,
                                        op=ALU.subtract)
                nc.sync.dma_start(out=out_v2[U], in_=ot)

    _split_excess_waits(nc)
    return nc


def _get_program(n_super, use_bias=False):
    key = (n_super, use_bias)
    if key not in _PROGRAMS:
        _PROGRAMS[key] = _build_program(n_super, use_bias)
    return _PROGRAMS[key]


# ---------------------------------------------------------------------------
# Entry point
# ---------------------------------------------------------------------------
def kernel(signal, conv1_w, conv1_b, conv2_w, conv2_b, fc1_w, fc1_b,
           fc2_w, fc2_b, pamap_w, pamap_b, **_unused):
    import ml_dtypes
    from concourse.bass_utils import run_bass_kernel_spmd

    n_super = _N_SUPER
    b_core = n_super * SUPER
    signal = np.asarray(signal, np.float32)
    b_tot = signal.shape[0]
    assert b_tot == b_core * NCORES, (b_tot, b_core)

    use_bias = any(np.any(np.asarray(b)) for b in
                   (conv1_b, conv2_b, fc1_b, fc2_b))
    b16, b32 = _build_blobs(np.asarray(conv1_w), np.asarray(conv1_b),
                            np.asarray(conv2_w), np.asarray(conv2_b),
                            np.asarray(fc1_w), np.asarray(fc1_b),
                            np.asarray(fc2_w), np.asarray(fc2_b),
                            np.asarray(pamap_w), np.asarray(pamap_b))

    nc = _get_program(n_super, use_bias)
    flat = signal.reshape(b_tot, F_IN)
    sig16 = flat.astype(ml_dtypes.bfloat16)
    pad = np.zeros((1, F_IN), ml_dtypes.bfloat16)

    in_maps = []
    for c in range(NCORES):
        sl = slice(c * b_core, (c + 1) * b_core)
        in_maps.append({
            "sig": np.concatenate([sig16[sl], pad], axis=0),
            "wb16": b16, "wb32": b32,
        })
    res = run_bass_kernel_spmd(nc, in_maps, core_ids=list(range(NCORES)))
    outs = [res.results[c]["out"] for c in range(NCORES)]
    return np.concatenate(outs, axis=0)


# revision 65
# speedup vs baseline: 1.1260x; 1.1260x over previous
"""Trainium2 Bass kernel for the ConvNet problem.

v3 design (all-bf16 matmul path, fp32 PSUM accumulation):
  - Host ships signal only, as bf16 [B+1, 150] (padded row).  Per
    super-tile (2048 samples) two DMA-XBAR transposes load feature-major
    bf16 tiles straight from DRAM: xA (feats 0..127) and xB (feats
    128..146 + harmless overrun, 32 rows).  x^2 tiles are computed
    on-device (DVE/Pool elementwise multiply) -- halves HBM traffic vs
    shipping signal^2 from host.
  - Windowed sums (banded matmuls), std = sqrt((s2 - s'^2)/9).
  - conv1 is emitted as three band-replicated output chunks (h1 rows
    0..74 / 60..139 / 125..189) so that each conv2 output chunk of 128
    features needs a single K-chunk: conv1 3 matmuls + conv2 3 matmuls
    (vs 2 + 6 for the dense 2-chunk layout).
  - fc1 (3 K-chunks x 2 M-chunks), fc2, pamap via weight streaming,
    log_softmax in a second phase (Exp/Ln table).
  - PSUM tags: pS / pS2 (1 bank each, reused by fc2 / pamap), pH1
    (3 banks, conv1 + fc1), pH2 (3 banks, conv2) = 8 banks total.
  - Drains are ReLUs spread across Act/DVE/Pool for engine balance
    (biases in this problem are structurally zero; a bias-correct
    fallback program is built if any bias is nonzero).

Sharding: pure data parallelism, batch split 8 ways across NeuronCores.
"""

import math
import os

import numpy as np

# ---------------------------------------------------------------------------
# Problem constants (hardcoded; kernel.py must be self-contained)
# ---------------------------------------------------------------------------
B_TOTAL, L, C = 131072, 50, 3
WIN = 10
NCORES = 8
B_CORE = B_TOTAL // NCORES          # 16384
G_SUPER = 16                        # samples per partition per super-tile
SUPER = 128 * G_SUPER               # 2048 samples per super-tile
N_SUPER_FULL = B_CORE // SUPER      # 8
NSUB = 256                          # samples per sub-tile (matmul N)
SUBQ = SUPER // NSUB                # 4 sub-tiles per super-tile
F_IN = L * C                        # 150
F_USE = 147                         # features actually consumed
F_STD = 120                         # 40 windows x 3 channels
F_C1 = 190                          # 38 x 5
F_C2 = 360                          # 36 x 10
F_FC1 = 256
F_FC2 = 64
F_OUT = 12

# conv1 band-replica chunks (h1 row ranges, in 190-row coordinates) chosen
# so conv2 output chunk m (features 128m..) reads h1 rows from replica m
# starting at partition 0.
C1_CH = ((0, 75), (60, 140), (125, 190))      # (lo, hi) -> rows lo..hi-1
C2_MO = (0, 128, 256, 360)                    # conv2 output chunk offsets
FC1_K = (128, 128, 104)                       # fc1 K-chunk sizes

# debug knob (harness never sets this; default = full problem)
_N_SUPER = int(os.environ.get("ATRN_NSUPER", str(N_SUPER_FULL)))


# ---------------------------------------------------------------------------
# Tile drain patch: walrus in this container rejects >2 sem waits on a
# CTRL-class (Drain) instruction.  Spread the end-of-kernel global-clock waits
# across per-proc SP nops (one sem each) before an unadorned drain.
# ---------------------------------------------------------------------------
def _install_drain_patch():
    import concourse.tile as tile
    from concourse.tile_scheduler import N_PROCS
    from concourse.vector_clock import ScopedClock, VectorClock

    if getattr(tile.TileContext, "_drain_patch_installed", False):
        return

    def _patched_drain_and_barrier(self, tick_clock, wait_clock):
        nc = self.nc
        gc = tick_clock.global_clock
        for p in range(N_PROCS):
            if gc[p] <= 0:
                continue
            v = [0] * N_PROCS
            v[p] = gc[p]
            nop = nc.sync.nop()
            wait_clock.add_sem_waits(nop.ins, ScopedClock({None: VectorClock(v)}))
        nc.sync.drain()
        nc.all_engine_barrier()
        assert self.sems is not None
        popped = nc._tile_sem_poison_stack.pop()
        assert popped is self._sem_poison
        nc.clear_and_free_semaphores(list(self.sems.allocated().values()))
        nc.all_engine_barrier()

    tile.TileContext._drain_and_barrier = _patched_drain_and_barrier
    tile.TileContext._drain_patch_installed = True


def _wait_cap(ins):
    # This walrus build rejects >1 sem wait on engine instructions.
    return 1


def _split_excess_waits(nc):
    """Hoist excess sem waits onto same-engine nops inserted just before."""
    from concourse import mybir

    ctr = 0
    for f in nc.m.functions:
        for blk in f.blocks:
            il = blk.instructions
            i = 0
            while i < len(il):
                ins = il[i]
                si = ins.sync_info
                cap = _wait_cap(ins)
                if si is not None and len(si.on_wait) > cap:
                    waits = list(si.on_wait)
                    extra, keep = waits[:-cap], waits[-cap:]
                    for w in extra:
                        ctr += 1
                        nop = mybir.InstNoOp(name=f"waitsplit-{ctr}",
                                             ins=[], outs=[])
                        nop.engine = ins.engine
                        nop.sync_info = type(si)(on_wait=[w], on_update=[])
                        nc.register_instruction(nop, overwrite=True)
                        il.insert(i, nop)
                        i += 1
                    ins.sync_info = type(si)(on_wait=keep,
                                             on_update=list(si.on_update))
                i += 1


# ---------------------------------------------------------------------------
# Host-side weight preprocessing
#   wb16: [128, W16] bf16 matmul weights
#   wb32: [128, W32] f32 biases (fallback variant) + bp for phase B
# ---------------------------------------------------------------------------
class _BlobLayout:
    def __init__(self):
        self.cols = 0
        self.slots = {}

    def add(self, name, rows, cols):
        self.slots[name] = (self.cols, rows, cols)
        self.cols += cols
        return self.slots[name]


_L16 = _BlobLayout()
_L16.add("sA_a", 128, F_STD)    # windowed-sum (scaled 1/sqrt(10)) rows 0..127
_L16.add("sA_b", 19, F_STD)     # rows 128..146
_L16.add("s2_a", 128, F_STD)    # unscaled (exact 1.0 in bf16)
_L16.add("s2_b", 19, F_STD)
for j in range(3):
    _L16.add(f"c1r{j}", F_STD, 80)  # conv1 band replica j (zero-padded to 80)
for m in range(3):
    lo, hi = C1_CH[m]
    _L16.add(f"c2m{m}", hi - lo, 128)  # chunk2 zero-padded 104->128
for k in range(3):
    for m in range(2):
        _L16.add(f"f1_{k}_{m}", FC1_K[k], 128)
for k in range(2):
    _L16.add(f"f2_{k}", 128, F_FC2)
_L16.add("wp", F_FC2, F_OUT)    # pamap rhs (weight-streaming)
W16 = _L16.cols

_L32 = _BlobLayout()
_L32.add("b1", 80, 3)           # conv1 bias per (partition, replica chunk)
_L32.add("b2", 128, 3)          # conv2 bias per (partition, chunk)
_L32.add("b3", 128, 2)          # fc1 bias
_L32.add("b4", F_FC2, 1)        # fc2 bias
_L32.add("bp", 128, F_OUT)      # pamap bias replicated across partitions
W32 = _L32.cols


def _build_blobs(conv1_w, conv1_b, conv2_w, conv2_b, fc1_w, fc1_b, fc2_w,
                 fc2_b, pamap_w, pamap_b):
    import ml_dtypes
    b16 = np.zeros((128, W16), ml_dtypes.bfloat16)
    b32 = np.zeros((128, W32), np.float32)

    def put16(name, arr):
        off, rows, cols = _L16.slots[name]
        assert arr.shape == (rows, cols), (name, arr.shape, (rows, cols))
        b16[:rows, off:off + cols] = arr.astype(ml_dtypes.bfloat16)

    def put32(name, arr):
        off, rows, cols = _L32.slots[name]
        assert arr.shape == (rows, cols), (name, arr.shape, (rows, cols))
        b32[:rows, off:off + cols] = arr.astype(np.float32)

    # windowed sums: s'[3l+c] = (1/sqrt(10)) * sum_k x[3(l+k)+c]
    A = np.zeros((F_USE, F_STD), np.float32)
    for m in range(F_STD):
        l, c = divmod(m, 3)
        for k in range(WIN):
            A[3 * (l + k) + c, m] = 1.0
    put16("sA_a", A[:128] / math.sqrt(10.0))
    put16("sA_b", A[128:] / math.sqrt(10.0))
    put16("s2_a", A[:128])
    put16("s2_b", A[128:])

    # conv1 as dense [in 120, out 190]
    M1 = np.zeros((F_STD, F_C1), np.float32)
    for t in range(38):
        for o in range(5):
            for k in range(3):
                for i in range(3):
                    M1[3 * (t + k) + i, 5 * t + o] = conv1_w[o, i, k]
    for j, (lo, hi) in enumerate(C1_CH):
        blk = np.zeros((F_STD, 80), np.float32)
        blk[:, :hi - lo] = M1[:, lo:hi]
        put16(f"c1r{j}", blk)

    # conv2 as dense [in 190, out 360]; chunk m reads h1 replica m
    M2 = np.zeros((F_C1, F_C2), np.float32)
    for t in range(36):
        for o in range(10):
            for k in range(3):
                for i in range(5):
                    M2[5 * (t + k) + i, 10 * t + o] = conv2_w[o, i, k]
    for m in range(3):
        lo, hi = C1_CH[m]
        blk = np.zeros((hi - lo, 128), np.float32)
        blk[:, :C2_MO[m + 1] - C2_MO[m]] = M2[lo:hi, C2_MO[m]:C2_MO[m + 1]]
        # band check: all nonzeros of these output columns live in rows lo..hi
        assert np.all(M2[:lo, C2_MO[m]:C2_MO[m + 1]] == 0)
        assert np.all(M2[hi:, C2_MO[m]:C2_MO[m + 1]] == 0)
        put16(f"c2m{m}", blk)

    # fc1 [360, 256]; K-chunks match conv2 output chunks (128/128/104)
    F1 = fc1_w.T.astype(np.float32)
    for k in range(3):
        for m in range(2):
            put16(f"f1_{k}_{m}",
                  F1[C2_MO[k]:C2_MO[k + 1], m * 128:(m + 1) * 128])
    F2 = fc2_w.T.astype(np.float32)          # [256, 64]
    for k in range(2):
        put16(f"f2_{k}", F2[k * 128:(k + 1) * 128])
    put16("wp", pamap_w.T.astype(np.float32))  # [64, 12]

    # biases (fallback variant only; bp always used in phase B)
    b1 = np.zeros(F_C1, np.float32)
    for t in range(38):
        for o in range(5):
            b1[5 * t + o] = conv1_b[o]
    b1p = np.zeros((80, 3), np.float32)
    for j, (lo, hi) in enumerate(C1_CH):
        b1p[:hi - lo, j] = b1[lo:hi]
    put32("b1", b1p)
    b2 = np.zeros(F_C2, np.float32)
    for t in range(36):
        for o in range(10):
            b2[10 * t + o] = conv2_b[o]
    b2p = np.zeros((128, 3), np.float32)
    for m in range(3):
        mo = C2_MO[m + 1] - C2_MO[m]
        b2p[:mo, m] = b2[C2_MO[m]:C2_MO[m + 1]]
    put32("b2", b2p)
    put32("b3", np.stack([fc1_b[:128], fc1_b[128:]], axis=1))
    put32("b4", fc2_b[:, None])
    put32("bp", np.tile(pamap_b.astype(np.float32)[None, :], (128, 1)))
    return b16, b32


# ---------------------------------------------------------------------------
# Bass program
# ---------------------------------------------------------------------------
_PROGRAMS = {}


def _w16(w, name):
    off, rows, cols = _L16.slots[name]
    return w[0:rows, off:off + cols]


def _w32(w, name):
    off, rows, cols = _L32.slots[name]
    return w[0:rows, off:off + cols]


def _build_program(n_super, use_bias):
    import contextlib

    import concourse.bass as bass
    import concourse.tile as tile
    from concourse import mybir

    _install_drain_patch()
    f32 = mybir.dt.float32
    bf16 = mybir.dt.bfloat16
    AF = mybir.ActivationFunctionType
    ALU = mybir.AluOpType

    b_core = n_super * SUPER
    nc = bass.Bass("TRN2", target_bir_lowering=False, debug=False,
                   num_devices=NCORES)
    # +1 padded row: the feats-128..159 transpose window of the last sample
    # runs into the pad row instead of out of bounds.
    sig = nc.dram_tensor("sig", [b_core + 1, F_IN], bf16,
                         kind="ExternalInput")
    wb16 = nc.dram_tensor("wb16", [128, W16], bf16, kind="ExternalInput")
    wb32 = nc.dram_tensor("wb32", [128, W32], f32, kind="ExternalInput")
    out = nc.dram_tensor("out", [b_core, F_OUT], f32, kind="ExternalOutput")

    with tile.TileContext(nc) as tc:
        with contextlib.ExitStack() as ctx:
            singles = ctx.enter_context(tc.tile_pool(name="singles", bufs=1))
            xsup = ctx.enter_context(tc.tile_pool(name="xsup", bufs=2))
            sbx = ctx.enter_context(tc.tile_pool(name="sbx", bufs=3))
            sbh = ctx.enter_context(tc.tile_pool(name="sbh", bufs=3))
            psA = ctx.enter_context(tc.tile_pool(name="psA", bufs=1,
                                                 space="PSUM"))
            outp = ctx.enter_context(tc.tile_pool(name="outp", bufs=2))

            w16 = singles.tile([128, W16], bf16)
            nc.sync.dma_start(out=w16, in_=wb16[:, :])
            w32 = singles.tile([128, W32], f32)
            nc.scalar.dma_start(out=w32, in_=wb32[:, :])
            lgpool = ctx.enter_context(tc.tile_pool(name="lgp", bufs=1))
            logits_all = lgpool.tile([128, n_super * G_SUPER * F_OUT], f32,
                                     name="logits_all", tag="lg")

            # column n of sub-tile q holds sample q*512 + j*128 + p after the
            # weight-streaming pamap matmul; phase-B g index is (q j).
            out_v = out.rearrange("(T q j p) o -> T p (q j) o",
                                  q=SUBQ, j=NSUB // 128, p=128)

            def relu_drain(eng, dst, src, bias=None):
                if bias is not None:
                    eng_v = nc.vector if eng is nc.scalar else eng
                    eng_v.tensor_scalar(out=dst, in0=src, scalar1=bias,
                                        scalar2=0.0, op0=ALU.add, op1=ALU.max)
                elif eng is nc.scalar:
                    nc.scalar.activation(out=dst, in_=src, func=AF.Relu)
                else:
                    eng.tensor_scalar_max(out=dst, in0=src, scalar1=0.0)

            import concourse.bass as bass_mod

            def tr_src(t, T, coff, width):
                full = t[:, :]
                return bass_mod.AP(
                    tensor=full.tensor, offset=T * SUPER * F_IN + coff,
                    ap=[[F_IN, SUPER], [1, width]])

            n_sub = n_super * SUBQ
            xtiles = {}                 # super index -> (xA, xB, qA, qB)
            state = {}                  # cycle -> dict of live tiles

            def tr_src_at(t, base, coff, width, n):
                full = t[:, :]
                return bass_mod.AP(
                    tensor=full.tensor, offset=base * F_IN + coff,
                    ap=[[F_IN, n], [1, width]])

            def emit_transposes(T):
                # XBAR transpose needs src free size to be a multiple of 128,
                # so xB reads a full 128-wide window (overrun lands in the
                # next sample / pad row; only rows 0..18 are consumed).
                xA = xsup.tile([128, SUPER], bf16, tag="xA")
                xB = xsup.tile([128, SUPER], bf16, tag="xB")
                # chunked for super 0 so the pipeline's first windows are not
                # gated on a whole-super transpose
                nchunk = 8 if T == 0 else 1
                cw = SUPER // nchunk
                for c in range(nchunk):
                    base = T * SUPER + c * cw
                    nc.sync.dma_start_transpose(
                        out=xA[:, c * cw:(c + 1) * cw],
                        in_=tr_src_at(sig, base, 0, 128, cw))
                    nc.sync.dma_start_transpose(
                        out=xB[:, c * cw:(c + 1) * cw],
                        in_=tr_src_at(sig, base, 128, 128, cw))
                xtiles[T] = (xA, xB)

            def emit_stage0(v):
                # windowed sums + std chain for cycle v
                xA, xB = xtiles[v // SUBQ]
                cs = slice((v % SUBQ) * NSUB, (v % SUBQ + 1) * NSUB)
                st = state[v] = {}
                # x^2 on-device, one sub-tile slice at a time (saves shipping
                # signal^2 over HBM without a serial whole-super bubble)
                qa = sbx.tile([128, NSUB], bf16, tag="qa")
                nc.gpsimd.tensor_mul(out=qa, in0=xA[:, cs], in1=xA[:, cs])
                qb = sbx.tile([32, NSUB], bf16, tag="qb")
                nc.gpsimd.tensor_mul(out=qb, in0=xB[0:32, cs],
                                     in1=xB[0:32, cs])
                # s and s2 packed in one 2-bank PSUM tile: the pS ring is
                # stage0-only, so next window's stage0 never waits on drains
                sps = psA.tile([128, 2, NSUB], f32, tag="pS")
                nc.tensor.matmul(sps[0:F_STD, 0], _w16(w16, "sA_a"),
                                 xA[:, cs], start=True, stop=False)
                nc.tensor.matmul(sps[0:F_STD, 0], _w16(w16, "sA_b"),
                                 xB[0:19, cs], start=False, stop=True)
                nc.tensor.matmul(sps[0:F_STD, 1], _w16(w16, "s2_a"), qa,
                                 start=True, stop=False)
                nc.tensor.matmul(sps[0:F_STD, 1], _w16(w16, "s2_b"),
                                 qb[0:19], start=False, stop=True)
                t_sb = sbx.tile([F_STD, NSUB], f32, tag="t")
                # hw verifier: only ONE non-scalar PSUM input per instruction,
                # so the square must be the single-input Act Square
                nc.scalar.activation(out=t_sb, in_=sps[0:F_STD, 0],
                                     func=AF.Square)
                u_sb = sbx.tile([F_STD, NSUB], f32, tag="u")
                nc.vector.tensor_sub(out=u_sb, in0=sps[0:F_STD, 1], in1=t_sb)
                std = sbx.tile([F_STD, NSUB], bf16, tag="std")
                nc.scalar.activation(out=std, in_=u_sb, func=AF.Sqrt,
                                     scale=1.0 / 9.0)
                st["std"] = std

            def emit_conv1(v):
                st = state[v]
                std = st.pop("std")
                h1_ps = psA.tile([128, 3, NSUB], f32, tag="pH1")
                for j in range(3):
                    nc.tensor.matmul(h1_ps[0:80, j],
                                     _w16(w16, f"c1r{j}"), std)
                h1 = sbh.tile([80, 3, NSUB], bf16, tag="h1")
                if use_bias:
                    for j in range(3):
                        relu_drain(nc.scalar, h1[:, j], h1_ps[0:80, j],
                                   _w32(w32, "b1")[:, j:j + 1])
                else:
                    relu_drain(nc.scalar, h1, h1_ps[0:80])
                st["h1"] = h1

            def emit_conv2(v):
                st = state[v]
                h1 = st.pop("h1")
                h2_ps = psA.tile([128, 3, NSUB], f32, tag="pH2")
                for m in range(3):
                    lo, hi = C1_CH[m]
                    nc.tensor.matmul(h2_ps[:, m], _w16(w16, f"c2m{m}"),
                                     h1[0:hi - lo, m])
                h2 = sbh.tile([128, 3, NSUB], bf16, tag="h2")
                if use_bias:
                    for m in range(3):
                        relu_drain(nc.vector, h2[:, m], h2_ps[:, m],
                                   _w32(w32, "b2")[:, m:m + 1])
                else:
                    relu_drain(nc.vector, h2, h2_ps)
                st["h2"] = h2

            def emit_fc1(v):
                st = state[v]
                h2 = st.pop("h2")
                h3_ps = psA.tile([128, 2, NSUB], f32, tag="pH3")
                for m in range(2):
                    for k in range(3):
                        nc.tensor.matmul(h3_ps[:, m],
                                         _w16(w16, f"f1_{k}_{m}"),
                                         h2[0:FC1_K[k], k], start=(k == 0),
                                         stop=(k == 2))
                h3 = sbh.tile([128, 2, NSUB], bf16, tag="h3")
                if use_bias:
                    for m in range(2):
                        relu_drain(nc.vector, h3[:, m], h3_ps[:, m],
                                   _w32(w32, "b3")[:, m:m + 1])
                else:
                    relu_drain(nc.scalar, h3[:, 0], h3_ps[:, 0])
                    relu_drain(nc.vector, h3[:, 1], h3_ps[:, 1])
                st["h3"] = h3

            def emit_tail(v):
                st = state.pop(v)
                h3 = st["h3"]
                # h4 and logits share one PSUM tile on the pH2 ring: the tail
                # gates conv2(v+3) (mid-pipeline) instead of stage0 (head).
                tp = psA.tile([128, 2, NSUB], f32, tag="pT", bufs=2)
                h4_ps = tp[0:F_FC2, 0]
                for k in range(2):
                    nc.tensor.matmul(h4_ps, _w16(w16, f"f2_{k}"),
                                     h3[:, k], start=(k == 0), stop=(k == 1))
                h4 = sbh.tile([F_FC2, NSUB], bf16, tag="h4")
                relu_drain(nc.vector, h4, h4_ps,
                           _w32(w32, "b4") if use_bias else None)
                nj = NSUB // 128
                lg_ps = tp[:, 1, 0:nj * F_OUT]
                for j in range(nj):
                    nc.tensor.matmul(
                        lg_ps[:, j * F_OUT:(j + 1) * F_OUT],
                        h4[:, j * 128:(j + 1) * 128], _w16(w16, "wp"),
                        start=True, stop=True)
                nc.vector.tensor_copy(
                    out=logits_all[:, v * nj * F_OUT:(v + 1) * nj * F_OUT],
                    in_=lg_ps)

            # software pipeline: stage k of cycle v-k all emitted in window v.
            # Old cycles come first so each engine's in-order queue leads with
            # work whose inputs finished in earlier windows.
            emit_transposes(0)
            for v in range(n_sub + 4):
                if v % SUBQ == 0 and (v // SUBQ) + 1 < n_super:
                    emit_transposes(v // SUBQ + 1)
                if v < n_sub:
                    emit_stage0(v)
                if 0 <= v - 1 < n_sub:
                    emit_conv1(v - 1)
                if 0 <= v - 2 < n_sub:
                    emit_conv2(v - 2)
                if 0 <= v - 3 < n_sub:
                    emit_fc1(v - 3)
                if 0 <= v - 4 < n_sub:
                    emit_tail(v - 4)

            # ---------- phase B: log-softmax (exp/ln table set) ----------
            tc.no_sync_barrier()
            PAIR = 2
            n_pg = PAIR * G_SUPER
            w_bp = _w32(w32, "bp")                       # [128, 12]
            bp3d = bass_mod.AP(tensor=w_bp.tensor, offset=w_bp.offset,
                               ap=[w_bp.ap[0], [0, n_pg], w_bp.ap[1]])
            out_v2 = out.rearrange("(U g p) o -> U p g o", g=n_pg, p=128)
            for U in range(n_super // PAIR):
                chunk = logits_all[:, U * n_pg * F_OUT:
                                   (U + 1) * n_pg * F_OUT]
                ch3 = chunk.rearrange("p (g o) -> p g o", o=F_OUT)
                lb = outp.tile([128, n_pg, F_OUT], f32, tag="lb")
                nc.gpsimd.tensor_tensor(out=lb, in0=ch3, in1=bp3d, op=ALU.add)
                e = outp.tile([128, n_pg, F_OUT], f32, tag="e")
                nc.scalar.activation(out=e, in_=lb, func=AF.Exp)
                ssum = outp.tile([128, n_pg], f32, tag="ss")
                nc.vector.tensor_reduce(out=ssum, in_=e,
                                        axis=mybir.AxisListType.X, op=ALU.add)
                lse = outp.tile([128, n_pg], f32, tag="lse")
                nc.scalar.activation(out=lse, in_=ssum, func=AF.Ln)
                lse3 = bass_mod.AP(tensor=lse.tensor, offset=lse.offset,
                                   ap=[lse.ap[0], lse.ap[1], [0, F_OUT]])
                ot = outp.tile([128, n_pg, F_OUT], f32, tag="ot")
                nc.vector.tensor_tensor(out=ot, in0=lb, in1=lse3,
                                        op=ALU.subtract)
                nc.sync.dma_start(out=out_v2[U], in_=ot)

    _split_excess_waits(nc)
    return nc


def _get_program(n_super, use_bias=False):
    key = (n_super, use_bias)
    if key not in _PROGRAMS:
        _PROGRAMS[key] = _build_program(n_super, use_bias)
    return _PROGRAMS[key]


# ---------------------------------------------------------------------------
# Entry point
# ---------------------------------------------------------------------------
def kernel(signal, conv1_w, conv1_b, conv2_w, conv2_b, fc1_w, fc1_b,
           fc2_w, fc2_b, pamap_w, pamap_b, **_unused):
    import ml_dtypes
    from concourse.bass_utils import run_bass_kernel_spmd

    n_super = _N_SUPER
    b_core = n_super * SUPER
    signal = np.asarray(signal, np.float32)
    b_tot = signal.shape[0]
    assert b_tot == b_core * NCORES, (b_tot, b_core)

    use_bias = any(np.any(np.asarray(b)) for b in
                   (conv1_b, conv2_b, fc1_b, fc2_b))
    b16, b32 = _build_blobs(np.asarray(conv1_w), np.asarray(conv1_b),
                            np.asarray(conv2_w), np.asarray(conv2_b),
                            np.asarray(fc1_w), np.asarray(fc1_b),
                            np.asarray(fc2_w), np.asarray(fc2_b),
                            np.asarray(pamap_w), np.asarray(pamap_b))

    nc = _get_program(n_super, use_bias)
    flat = signal.reshape(b_tot, F_IN)
    sig16 = flat.astype(ml_dtypes.bfloat16)
    pad = np.zeros((1, F_IN), ml_dtypes.bfloat16)

    in_maps = []
    for c in range(NCORES):
        sl = slice(c * b_core, (c + 1) * b_core)
        in_maps.append({
            "sig": np.concatenate([sig16[sl], pad], axis=0),
            "wb16": b16, "wb32": b32,
        })
    res = run_bass_kernel_spmd(nc, in_maps, core_ids=list(range(NCORES)))
    outs = [res.results[c]["out"] for c in range(NCORES)]
    return np.concatenate(outs, axis=0)
